# revision 64
# baseline (speedup 1.0000x reference)
"""Trainium2 Bass kernel for nn_Attention_43190191129190.

Model (per batch element b of 8):
    y   = x + dwconv3x3(x) + conv_b          (depthwise residual positional conv)
    qkv = y @ qkv_w.T ; split into q, k, v   (8 heads, dim 32)
    out = softmax(q k^T / sqrt(32)) v
    out = out @ out_w.T + out_b

Sharding: pure data-parallel, one batch element per NeuronCore (8 cores).

Per-core design (transposed [C, N] space; see kernel_baseline.py.bak for the
ancestor).  v2 changes vs the baseline:

  * Attention is split over the QUERY dim into two 512-token halves.  All 4
    head pairs run on half 0, whose normalization + out-projection + store
    overlap half 1's attention; only half 1's last pair remains in the tail.
  * PSUM re-plan: pst 2x[128,1024] (S^T double-buffer), ppv 1x[128,1024]
    (PV accumulator, freed by a single DVE evacuation), paux 1x[128,1024]
    (conv ct0 / v / qk tiles 1,3 / softmax-broadcast / projection).
  * Softmax normalization with no DRAM round trip: evacuate pv -> pc (SBUF),
    PE-broadcast the sums row (ones stationary), DVE reciprocal from PSUM,
    DVE multiply; deferred past the next pair's first S^T so PE never waits.
  * ScalarE runs exps only (one [128,1024] exp per pair/m covering both
    heads); all other evacuations are on DVE (preamble ones may use ScalarE
    while it is still idle).
  * Input DMAs spread over the four HWDGE queues + the Pool SWDGE path.
"""

import os

import numpy as np

import concourse.bass as bass
import concourse.tile as tile
from concourse import bacc, mybir
from concourse.bass_utils import run_bass_kernel_spmd

F32 = mybir.dt.float32
F32R = mybir.dt.float32r
AF = mybir.ActivationFunctionType

B, N, C = 8, 1024, 256
HEADS, DH = 8, 32
SCALE = DH ** -0.5
PAD = 34  # 32x32 spatial grid with 1-px halo
HALF = 512

TAPS = [(ky, kx) for ky in range(3) for kx in range(3)]
# Head pairs per query half.  Half 0 must order so q/k feature tiles 1,3
# (emitted during its pair 1) precede any head >= 4.  Half 1 re-pairs so the
# final pair (0,4) writes attn^T rows 0:32 of both chunks directly -- no
# repositioning DMA in the kernel tail.
PAIRS_H = [
    [(1, 3), (0, 2), (5, 7), (4, 6)],
    [(1, 3), (5, 7), (2, 6), (0, 4)],
]


def build_nc(debug_dump=False):
    nc = bacc.Bacc("TRN2", target_bir_lowering=False, debug=False, num_devices=8)

    x_d = nc.dram_tensor("x", (N, C), F32R, kind="ExternalInput").ap()
    qkvwT_d = nc.dram_tensor("qkv_wT", (C, 3 * C), F32R, kind="ExternalInput").ap()
    outwT_d = nc.dram_tensor("out_wT", (C, C), F32R, kind="ExternalInput").ap()
    taps_d = nc.dram_tensor("conv_taps", (128, 18), F32, kind="ExternalInput").ap()
    convb_d = nc.dram_tensor("conv_b_r", (1, C), F32R, kind="ExternalInput").ap()
    outb_d = nc.dram_tensor("out_b_r", (1, C), F32R, kind="ExternalInput").ap()
    id_d = nc.dram_tensor("id128", (128, 128), F32R, kind="ExternalInput").ap()
    out_d = nc.dram_tensor("out", (N, C), F32, kind="ExternalOutput").ap()
    dbg = {}
    if debug_dump:
        for name, shape in (
            ("d_yT", (128, 2, N)), ("d_qT", (128, 2, N)), ("d_kT", (128, 2, N)),
            ("d_v", (128, 8, 8 * 33)), ("d_attnT", (128, 2, N)),
        ):
            dbg[name] = nc.dram_tensor(name, shape, F32, kind="ExternalOutput").ap()

    with tile.TileContext(nc) as tc:
        with (
            tc.tile_pool(name="const", bufs=1) as const,
            tc.tile_pool(name="xin", bufs=1) as xin_p,
            tc.tile_pool(name="big", bufs=1) as big,
            tc.tile_pool(name="pT", bufs=8) as ppool,
            tc.tile_pool(name="rs", bufs=2) as rs_p,
            tc.tile_pool(name="tmp", bufs=2) as tmp_p,
            tc.tile_pool(name="tmp2", bufs=2) as tmp2_p,
            tc.tile_pool(name="outs", bufs=3) as outs_p,
            tc.tile_pool(name="pst", bufs=2, space="PSUM") as pst,
            tc.tile_pool(name="ppv", bufs=1, space="PSUM") as ppv,
            tc.tile_pool(name="paux", bufs=1, space="PSUM") as paux,
        ):
            # ---- input DMAs.  Per-DMA SEQ dispatch (~1.26us) serializing
            # per queue is the startup bound, so x comes in THREE combined
            # DMAs and the q/k/v weights follow on the same sync queue; the
            # conv diagonals are built on-device from a 9KB tap vector
            # (identity x per-partition scale) instead of a 1.2MB DMA.  The
            # small loads ride the parallel Pool SWDGE path.
            taps_sb = const.tile([128, 18], F32, tag="taps")
            nc.gpsimd.dma_start(taps_sb, taps_d)
            id_sb = const.tile([128, 128], F32R, tag="id")
            nc.sync.dma_start(id_sb, id_d)
            diag_sb = const.tile([128, 18, 128], F32R, tag="diag")
            xg = [None] * 3
            for g, (r0, ntile) in enumerate(((0, 2), (256, 3), (640, 3))):
                xg[g] = xin_p.tile([128, ntile, C], F32R, tag=f"xg{g}",
                                   name=f"xg{g}")
                nc.sync.dma_start(
                    xg[g],
                    x_d[r0:r0 + ntile * 128, :].rearrange(
                        "(t p) c -> p t c", p=128),
                )
            xins = [xg[0][:, 0, :], xg[0][:, 1, :],
                    xg[1][:, 0, :], xg[1][:, 1, :], xg[1][:, 2, :],
                    xg[2][:, 0, :], xg[2][:, 1, :], xg[2][:, 2, :]]
            qkvwT_sb = const.tile([128, 2, 3 * C], F32R, tag="qkvwT")
            qkvw_r = qkvwT_d.rearrange("(kc p) f -> p kc f", p=128)
            nc.sync.dma_start(qkvwT_sb[:, :, 0:512], qkvw_r[:, :, 0:512])
            nc.sync.dma_start(qkvwT_sb[:, :, 512:768], qkvw_r[:, :, 512:768])
            convb_sb = const.tile([1, C], F32R, tag="convb")
            nc.gpsimd.dma_start(convb_sb, convb_d)
            ones_sb = const.tile([1, N], F32R, tag="ones")
            nc.gpsimd.memset(ones_sb.bitcast(mybir.dt.uint32), 0x3F800000)
            outwT_sb = const.tile([128, 2, C], F32R, tag="outwT")
            nc.gpsimd.dma_start(outwT_sb, outwT_d.rearrange("(kc p) f -> p kc f", p=128))
            outb_sb = const.tile([1, C], F32R, tag="outb")
            nc.gpsimd.dma_start(outb_sb, outb_d)
            zerob_sb = const.tile([128, 1], F32, tag="zerob")
            nc.vector.memset(zerob_sb, 0.0)
            # dummy exp: hoists the ~1.3us exp_and_others ACT table load into
            # the idle startup window (the set also contains Copy, so the
            # preamble ScalarE copies share it)
            warm_sb = const.tile([1, 1], F32, tag="warm")
            nc.scalar.activation(
                warm_sb, zerob_sb[0:1, 0:1], AF.Exp,
                bias=zerob_sb[0:1], scale=1.0,
            )
            # conv diag ct0 built on ScalarE: diag[:, i, :] = id * tap_i[p]
            # (per-partition scale); ct1 is built on DVE once the transpose
            # evacuations are queued (see below)
            for i in range(9):
                nc.scalar.mul(diag_sb[:, i, :], id_sb, taps_sb[:, i:i + 1])
            # all-ones strip on every partition (PE broadcast stationary must
            # share its base partition with the moving operand)
            onesp_sb = const.tile([128, 32], F32R, tag="onesp")
            nc.gpsimd.memset(onesp_sb.bitcast(mybir.dt.uint32), 0x3F800000)

            # PE p-state warm-up: a stream of tiny matmuls keeps the tensor
            # engine continuously busy from ~0.8us so the 3us ramp completes
            # before the first real transpose (which then runs at full rate)
            pdum = pst.tile([128, 1024], F32, tag="ps", name="pdum")
            zb = zerob_sb.bitcast(F32R)
            for i in range(40):
                nc.tensor.matmul(
                    pdum[0:1, 0:32],
                    lhsT=zb[0:1, 0:1],
                    rhs=onesp_sb[0:1, :],
                    start=(i == 0),
                    stop=(i == 39),
                )

            # ---- persistent activations ----
            xpadT = big.tile([128, 2, PAD * PAD], F32R, tag="xpadT")
            # zero only the 1-px halo ring (interior is fully overwritten)
            xpv = xpadT.bitcast(mybir.dt.uint32).rearrange(
                "p ct (h w) -> p ct h w", h=PAD
            )
            nc.gpsimd.memset(xpv[:, :, 0, :], 0)
            nc.gpsimd.memset(xpv[:, :, PAD - 1, :], 0)
            nc.gpsimd.memset(xpv[:, :, :, 0], 0)
            nc.gpsimd.memset(xpv[:, :, :, PAD - 1], 0)
            yT = big.tile([128, 2, N], F32R, tag="yT")
            qT = big.tile([128, 2, N], F32R, tag="qT")
            kT = big.tile([128, 2, N], F32R, tag="kT")
            vsb = big.tile([128, 8, 8 * 33], F32R, tag="v")
            # 1.0 everywhere (ones columns); v cols overwritten below
            nc.gpsimd.memset(vsb.bitcast(mybir.dt.uint32), 0x3F800000)
            attnT = big.tile([128, 2, N], F32R, tag="attnT")

            # preamble psum evacuations alternate between DVE and the (still
            # idle) ScalarE so neither queue gates slot turnover
            _cp = [0]

            def copy_alt(dst, src_ap):
                _cp[0] += 1
                if _cp[0] % 2:
                    nc.vector.tensor_copy(dst, src_ap)
                else:
                    nc.scalar.copy(dst, src_ap)

            # ---- transpose x into padded x^T, conv interleaved ----
            def emit_transpose(nt):
                tp = pst.tile([128, 1024], F32, tag="ps", name="tp")
                tpr = tp.bitcast(F32R)
                for ct in range(2):
                    nc.tensor.transpose(
                        tpr[:, 512 * ct: 512 * ct + 128],
                        xins[nt][:, 128 * ct: 128 * (ct + 1)],
                        id_sb,
                    )
                    dst = xpadT[:, ct, :].rearrange("p (h w) -> p h w", h=PAD)[
                        :, 1 + 4 * nt: 5 + 4 * nt, 1:33
                    ]
                    nc.vector.tensor_copy(
                        dst,
                        tp[:, 512 * ct: 512 * ct + 128].rearrange(
                            "p (a b) -> p a b", a=4
                        ),
                    )

            # conv accumulators: ct0 in the aux psum slot, ct1 in the (still
            # idle) PV slot, so transposes keep both pst slots
            cacc = [paux.tile([128, 1024], F32, tag="aux", name="cacc0"),
                    ppv.tile([128, 1024], F32, tag="pv", name="cacc1")]

            def emit_conv_part(ct, j, t0, t1, bias=False):
                cps = cacc[ct]
                view = xpadT[:, ct, :].rearrange("p (h w) -> p h w", h=PAD)
                for t in range(t0, t1):
                    ky, kx = TAPS[t]
                    nc.tensor.matmul(
                        cps[:, j * 512:(j + 1) * 512],
                        lhsT=diag_sb[:, ct * 9 + t, :],
                        rhs=view[:, ky + 16 * j: ky + 16 * j + 16, kx: kx + 32],
                        start=(t == 0),
                        stop=False,
                    )
                if bias:
                    nc.tensor.matmul(
                        cps[:, j * 512:(j + 1) * 512],
                        lhsT=convb_sb[0:1, 128 * ct: 128 * (ct + 1)],
                        rhs=ones_sb[0:1, j * 512:(j + 1) * 512],
                        start=False,
                        stop=True,
                    )

            def emit_conv_half(ct, j):
                emit_conv_part(ct, j, 0, 9, bias=True)

            # conv j=0 only needs padded rows 0..18 (x tiles 0..4) and
            # transposes 5..7 only feed conv j=1, so the preamble critical
            # path is transposes 0-4 -> conv(ct,0) -> q/k j=0.  conv j=1
            # (query tokens 512..1023, first needed at m-step 4 of pair 0)
            # is interleaved into pair 0's m-loop below.  The q/k j=0
            # contraction is split by feature chunk: the ct0 matmuls run
            # while conv ct1 still waits for its diag DMA.
            for nt in range(5):
                emit_transpose(nt)
            emit_conv_half(0, 0)
            for i in range(9, 18):
                nc.scalar.mul(diag_sb[:, i, :], id_sb, taps_sb[:, i:i + 1])
            nc.vector.tensor_copy(yT[:, 0, 0:512], cacc[0][:, 0:512])
            for nt in range(5, 8):
                emit_transpose(nt)
            qk_ps = {}
            for ft in (0, 2):
                qk_ps[ft] = pst.tile([128, 1024], F32, tag="ps", name="qkps")
                nc.tensor.matmul(
                    qk_ps[ft][:, 0:512],
                    lhsT=qkvwT_sb[:, 0, (0 if ft < 2 else 256): (0 if ft < 2 else 256) + 128],
                    rhs=yT[:, 0, 0:512],
                    start=True,
                    stop=False,
                )
            emit_conv_half(1, 0)
            nc.scalar.copy(yT[:, 1, 0:512], cacc[1][:, 0:512])
            for ft, dstT in ((0, qT), (2, kT)):
                nc.tensor.matmul(
                    qk_ps[ft][:, 0:512],
                    lhsT=qkvwT_sb[:, 1, (0 if ft < 2 else 256): (0 if ft < 2 else 256) + 128],
                    rhs=yT[:, 1, 0:512],
                    start=False,
                    stop=True,
                )
                nc.vector.tensor_copy(dstT[:, 0, 0:512], qk_ps[ft][:, 0:512])

            # ---- q^T / k^T feature tiles.  Tiles 0,2 (heads 0-3): the j=0
            # token halves run in the preamble, j=1 inside pair 0's m-loop.
            # Tiles 1,3 (heads 4-7) run inside pair 1's m-loop. ----
            def emit_qk(ft, pool, js=(0, 1), eng=None):
                dstT, dc = (qT, ft) if ft < 2 else (kT, ft - 2)
                fofs = 0 if ft < 2 else 256
                qps = pool.tile([128, 1024], F32, tag="ps" if pool is pst else "aux",
                                name="qps")
                for j in js:
                    for kc in range(2):
                        nc.tensor.matmul(
                            qps[:, j * 512:(j + 1) * 512],
                            lhsT=qkvwT_sb[:, kc, fofs + dc * 128: fofs + (dc + 1) * 128],
                            rhs=yT[:, kc, j * 512:(j + 1) * 512],
                            start=(kc == 0),
                            stop=(kc == 1),
                        )
                for j in js:
                    (eng or nc.vector).tensor_copy(
                        dstT[:, dc, j * 512:(j + 1) * 512],
                        qps[:, j * 512:(j + 1) * 512],
                    )

            def emit_v(nt):
                vps = paux.tile([128, 1024], F32, tag="aux", name="vps")
                for kc in range(2):
                    nc.tensor.matmul(
                        vps[:, 0:256],
                        lhsT=yT[:, kc, nt * 128:(nt + 1) * 128],
                        rhs=qkvwT_sb[:, kc, 512:768],
                        start=(kc == 0),
                        stop=(kc == 1),
                    )
                vv = vsb[:, nt, :].rearrange("p (hh c) -> p hh c", c=33)
                sv = vps[:, 0:256].rearrange("p (hh c) -> p hh c", c=32)
                nc.vector.tensor_copy(vv[:, :, 0:32], sv)  # [v_h | 1] per head

            emit_qk(0, pst, js=(0,))
            emit_qk(2, pst, js=(0,))

            # ---- out-projection of one 128-token tile (all 8 heads) ----
            # stores ride sync + the Pool SWDGE path: a scalar-queue store
            # would inject a ~1.3us DMA dispatch into the ACT SEQ mid-exp
            _oq = [nc.sync, nc.gpsimd, nc.sync, nc.gpsimd]

            def emit_proj(nt, pool=None):
                pool = pool or paux
                ops = pool.tile([128, 1024], F32,
                                tag="ps" if pool is pst else "aux", name="ops")
                for chunk in range(2):
                    nc.tensor.matmul(
                        ops[:, 0:256],
                        lhsT=attnT[:, chunk, nt * 128:(nt + 1) * 128],
                        rhs=outwT_sb[:, chunk, :],
                        start=(chunk == 0),
                        stop=False,
                    )
                nc.tensor.matmul(
                    ops[:, 0:256],
                    lhsT=ones_sb[0:1, 0:128],
                    rhs=outb_sb,
                    start=False,
                    stop=True,
                )
                osb = outs_p.tile([128, C], F32, tag="o")
                nc.vector.tensor_copy(osb, ops[:, 0:256])
                _oq[nt % 4].dma_start(out_d[nt * 128:(nt + 1) * 128, :], osb)

            # ---- attention: two query halves x four head pairs ----
            # aux work interleaved into the m-loops, a slice per m-step:
            #   (half0, pair0): conv j=1 + its yT evacs, q/k tile-0 j=1
            #                   slices, v(0..7) (v(m) must precede PV(m))
            #   (half0, pair1): qk tiles 1,3 (needed by pair 2 = heads 5,7)
            #   (half1, pair0/1): projection of half-0 tiles 0..3
            def pair_extra(half, ip, m):
                if half == 0 and ip == 0:
                    if m == 0:
                        emit_conv_part(0, 1, 0, 5)
                    elif m == 1:
                        emit_conv_part(0, 1, 5, 9, bias=True)
                        nc.vector.tensor_copy(yT[:, 0, 512:1024],
                                              cacc[0][:, 512:1024])
                    elif m == 2:
                        emit_conv_part(1, 1, 0, 5)
                        emit_v(0)
                    elif m == 3:
                        emit_conv_part(1, 1, 5, 9, bias=True)
                        nc.vector.tensor_copy(yT[:, 1, 512:1024],
                                              cacc[1][:, 512:1024])
                        emit_qk(2, paux, js=(1,))
                        emit_v(1)
                    elif m == 4:
                        emit_v(2)
                    elif m == 5:
                        emit_v(3)
                        emit_v(4)
                    elif m == 6:
                        emit_v(5)
                        emit_v(6)
                    elif m == 7:
                        emit_v(7)
                        emit_qk(0, paux, js=(1,))
                elif half == 0 and ip == 1:
                    # only the j=0 half of q tile 1 is needed before half 1;
                    # k tile 3 needs both halves by pair 2's m-step 4
                    if m == 3:
                        emit_qk(1, paux, js=(0,))
                    elif m == 5:
                        emit_qk(3, paux, js=(0,))
                    elif m == 6:
                        emit_qk(3, paux, js=(1,))
                elif half == 1 and ip == 0:
                    if m == 1:
                        emit_qk(1, paux, js=(1,))
                    elif m in (5, 7):
                        emit_proj((m - 5) // 2)
                elif half == 1 and ip == 1 and m in (1, 5):
                    emit_proj(2 + (m - 1) // 4)
                elif half == 1 and ip == 3 and m >= 4:
                    # partial projection of half-1 tiles: contributions from
                    # attn^T rows 32:64 and 96:128 (normalized well before
                    # this last pair -- rows 64:96 depend on pair (2,6)'s
                    # repositioning DMAs and join in the tail) plus the bias,
                    # staged to SBUF.
                    opsp = paux.tile([128, 1024], F32, tag="aux", name="opsp")
                    for chunk in range(2):
                        for a, k in ((32, 32), (64, 64)):
                            nc.tensor.matmul(
                                opsp[:, 0:256],
                                lhsT=attnT[a:a + k, chunk, m * 128:(m + 1) * 128],
                                rhs=outwT_sb[a:a + k, chunk, :],
                                start=(chunk == 0 and a == 32),
                                stop=False,
                                tile_position=(a, 0),
                            )
                    nc.tensor.matmul(
                        opsp[:, 0:256],
                        lhsT=ones_sb[0:1, 0:128],
                        rhs=outb_sb,
                        start=False,
                        stop=True,
                    )
                    if m < 7:
                        nc.vector.tensor_copy(partial4[:, m - 4, :],
                                              opsp[:, 0:256])
                    else:
                        opsp7[0] = opsp

            def emit_norm_b(half, hA, hB, pc, pool=None):
                # PE-broadcast the sums row (pc row 32) to 32 partitions,
                # reciprocal straight from psum, then normalize.
                pool = pool or paux
                bcp = pool.tile([128, 1024], F32,
                                tag="ps" if pool is pst else "aux", name="bcp")
                for j in range(2):
                    nc.tensor.matmul(
                        bcp[0:32, j * 512:(j + 1) * 512],
                        lhsT=onesp_sb[32:33, :],
                        rhs=pc[32:33, j * 512:(j + 1) * 512],
                        start=True,
                        stop=True,
                    )
                rs = rs_p.tile([128, 1024], F32, tag="rs")
                nc.vector.reciprocal(rs[0:32, :], bcp[0:32, :])
                pcb = pc.bitcast(F32)
                nofs = half * HALF
                for hd, h in ((0, hA), (1, hB)):
                    row = 32 * (h % 4)
                    hc = h // 4
                    cofs = hd * 512
                    if row == 0:
                        nc.vector.tensor_mul(
                            attnT[0:32, hc, nofs:nofs + HALF],
                            pcb[0:32, cofs:cofs + 512],
                            rs[0:32, cofs:cofs + 512],
                        )
                    else:
                        # reposition to the head's attn^T rows (DMA can shift
                        # partitions; DVE cannot)
                        pcs = tmp2_p.tile([128, 512], F32R, tag="pcs", name="pcs")
                        nc.vector.tensor_mul(
                            pcs[0:32, :],
                            pcb[0:32, cofs:cofs + 512],
                            rs[0:32, cofs:cofs + 512],
                        )
                        nc.sync.dma_start(
                            attnT[row:row + 32, hc, nofs:nofs + HALF], pcs[0:32, :]
                        )

            # The PV lag list is carried ACROSS pair boundaries: the trailing
            # PV accumulations of pair p are emitted during the first m-steps
            # of pair p+1, so the ScalarE exp stream never waits on a burst
            # of trailing PVs.  pv psum tiles are allocated lazily (at the
            # m==0 PV) so pair 0's conv-j1 accumulators can share the slots;
            # pair 0 uses lag 3 since v(0) only exists from its m-step 2.
            # The pv evacuation (one DVE copy) follows the m==7 PV; the rest
            # of the normalization lands >= m-step 3 of the next pair, well
            # past any PE dependency.
            pend = []  # (pid, half, hA, hB, m, pT) awaiting PV matmuls
            pending_norm = None  # (half, hA, hB, pc)
            pvt = {}  # pid -> lazily allocated pv psum tile
            partial4 = big.tile([128, 4, C], F32R, tag="partial4")
            opsp7 = [None]  # last partial-proj psum, evacuated in the tail

            def emit_pv(pid, half, hA, hB, m, pT):
                nonlocal pending_norm
                if m == 0:
                    pvt[pid] = ppv.tile([128, 1024], F32, tag="pv", name="pv")
                pv = pvt[pid]
                for hd, h in ((0, hA), (1, hB)):
                    nc.tensor.matmul(
                        pv[0:33, hd * 512:(hd + 1) * 512],
                        lhsT=vsb[:, m, 33 * h: 33 * h + 33],
                        rhs=pT[:, hd * 512:(hd + 1) * 512],
                        start=(m == 0),
                        stop=(m == 7),
                    )
                if m == 7:
                    pc = tmp_p.tile([128, 1024], F32R, tag="pc", name="pc")
                    nc.vector.tensor_copy(pc[0:33, :], pv[0:33, :])
                    pending_norm = (half, hA, hB, pc)
                    del pvt[pid]

            for half in range(2):
                for ip, (hA, hB) in enumerate(PAIRS_H[half]):
                    pid = half * 4 + ip
                    lag = 3 if pid == 0 else 2
                    for m in range(8):
                        st = pst.tile([128, 1024], F32, tag="ps")
                        for hd, h in ((0, hA), (1, hB)):
                            a = 32 * (h % 4)
                            hc = h // 4
                            nc.tensor.matmul(
                                st[:, hd * 512:(hd + 1) * 512],
                                lhsT=kT[a:a + 32, hc, m * 128:(m + 1) * 128],
                                rhs=qT[a:a + 32, hc, half * HALF: half * HALF + 512],
                                start=True,
                                stop=True,
                                tile_position=(a, 0),
                            )
                        # drain up to two prev-pair PVs per step (so the pv
                        # evacuation lands >= 2 steps before this pair's m==0
                        # PV reuses the psum slot), plus one own-pair PV once
                        # past the lag
                        for _ in range(2):
                            if pend and pend[0][0] != pid:
                                emit_pv(*pend.pop(0))
                        if pend and pend[0][0] == pid and len(pend) > lag:
                            emit_pv(*pend.pop(0))
                        if pending_norm is not None and m >= 2:
                            emit_norm_b(*pending_norm)
                            pending_norm = None
                        pT = ppool.tile([128, 1024], F32R, tag="pT")
                        nc.scalar.activation(pT, st, AF.Exp, bias=zerob_sb, scale=SCALE)
                        pair_extra(half, ip, m)
                        pend.append((pid, half, hA, hB, m, pT))
            # ---- tail: drain the PV backlog; the last pair's (m==7) PVs are
            # followed per-head by a short normalize chain (copy, broadcast,
            # reciprocal, multiply -- heads 0 and 4 both land on rows 0:32 so
            # there is no repositioning), then the half-1 projection finishes
            # with the two K=32 row-0 contributions per tile and stores.
            while len(pend) > 1:
                emit_pv(*pend.pop(0))
            lpid, lhalf, lhA, lhB, lm, lpT = pend.pop(0)
            pv = pvt[lpid]
            nofs = lhalf * HALF
            for hd, h in ((0, lhA), (1, lhB)):
                nc.tensor.matmul(
                    pv[0:33, hd * 512:(hd + 1) * 512],
                    lhsT=vsb[:, lm, 33 * h: 33 * h + 33],
                    rhs=lpT[:, hd * 512:(hd + 1) * 512],
                    start=False,
                    stop=True,
                )
            # evacuate the two heads' unnormalized tiles + sums, one on the
            # (now idle) ScalarE and one on DVE so they overlap; the deferred
            # m==7 partial-proj evacuation is emitted AFTER the reciprocal so
            # the normalize chain jumps ahead of it in the DVE queue.
            pc2 = tmp_p.tile([128, 1024], F32R, tag="pc", name="pc2")
            nc.scalar.copy(pc2[0:33, 0:512], pv[0:33, 0:512])
            nc.scalar.copy(pc2[0:33, 512:1024], pv[0:33, 512:1024])
            # broadcast both heads' sums, ONE reciprocal over the combined
            # rows, then normalize (both heads land on rows 0:32)
            bcp2 = pst.tile([128, 1024], F32, tag="ps", name="bcp2")
            for hd in range(2):
                nc.tensor.matmul(
                    bcp2[0:32, hd * 512:(hd + 1) * 512],
                    lhsT=onesp_sb[32:33, :],
                    rhs=pc2[32:33, hd * 512:(hd + 1) * 512],
                    start=True,
                    stop=True,
                )
            rs2t = rs_p.tile([128, 1024], F32, tag="rs", name="rs2t")
            nc.vector.reciprocal(rs2t[0:32, :], bcp2[0:32, :])
            nc.vector.tensor_copy(partial4[:, 3, :], opsp7[0][:, 0:256])
            pc2b = pc2.bitcast(F32)
            for hd, h in ((0, lhA), (1, lhB)):
                nc.vector.tensor_mul(
                    attnT[0:32, h // 4, nofs:nofs + HALF],
                    pc2b[0:32, hd * 512:(hd + 1) * 512],
                    rs2t[0:32, hd * 512:(hd + 1) * 512],
                )

            if debug_dump:
                nc.sync.dma_start(dbg["d_yT"], yT.bitcast(F32))
                nc.sync.dma_start(dbg["d_qT"], qT.bitcast(F32))
                nc.sync.dma_start(dbg["d_kT"], kT.bitcast(F32))
                nc.sync.dma_start(dbg["d_v"], vsb.bitcast(F32))
                nc.sync.dma_start(dbg["d_attnT"], attnT.bitcast(F32))

            # per psum tile (two output tiles each, separate banks): fold
            # the staged partial via an identity matmul, add the row-0
            # contributions, evacuate + store per half so the A store
            # launches while B's matmuls still run
            opsf = [pst.tile([128, 1024], F32, tag="ps", name="opsfA"),
                    paux.tile([128, 1024], F32, tag="aux", name="opsfB")]
            for half_t, q in ((0, nc.sync), (1, nc.scalar)):
                for i in (0, 1):
                    nt = 4 + half_t * 2 + i
                    sl = opsf[half_t][:, i * 512:i * 512 + 256]
                    nc.tensor.matmul(
                        sl,
                        lhsT=id_sb,
                        rhs=partial4[:, half_t * 2 + i, :],
                        start=True,
                        stop=False,
                    )
                    for chunk in range(2):
                        nc.tensor.matmul(
                            sl,
                            lhsT=attnT[0:32, chunk, nt * 128:(nt + 1) * 128],
                            rhs=outwT_sb[0:32, chunk, :],
                            start=False,
                            stop=(chunk == 1),
                            tile_position=(0, 0),
                        )
                osb2 = outs_p.tile([128, 2, C], F32, tag="o", name="osb2")
                osrc = opsf[half_t].rearrange("p (t c) -> p t c", t=2)[:, :, 0:256]
                if half_t == 0:
                    nc.vector.tensor_copy(osb2, osrc)
                else:
                    nc.scalar.copy(osb2, osrc)
                r0 = 512 + half_t * 256
                q.dma_start(
                    out_d[r0:r0 + 256, :].rearrange("(t p) c -> p t c", p=128),
                    osb2,
                )

    nc.compile()
    return nc


_NC = None
LAST_RESULTS = None


def _host_prep(conv_w, conv_b, qkv_w, out_w, out_b):
    conv_w = np.asarray(conv_w, np.float32).reshape(C, 3, 3)
    taps = np.zeros((128, 18), np.float32)
    for ct in range(2):
        for t, (ky, kx) in enumerate(TAPS):
            d = conv_w[128 * ct: 128 * (ct + 1), ky, kx].copy()
            if (ky, kx) == (1, 1):
                d += 1.0  # residual connection folded into the center tap
            taps[:, ct * 9 + t] = d
    return {
        "qkv_wT": np.ascontiguousarray(np.asarray(qkv_w, np.float32).T),
        "out_wT": np.ascontiguousarray(np.asarray(out_w, np.float32).T),
        "conv_taps": taps,
        "conv_b_r": np.asarray(conv_b, np.float32).reshape(1, C),
        "out_b_r": np.asarray(out_b, np.float32).reshape(1, C),
        "id128": np.eye(128, dtype=np.float32),
    }


def kernel(x, conv_w, conv_b, qkv_w, out_w, out_b):
    global _NC, LAST_RESULTS
    if _NC is None:
        _NC = build_nc()
    x = np.asarray(x, np.float32)
    shared = _host_prep(conv_w, conv_b, qkv_w, out_w, out_b)
    in_maps = [{**shared, "x": np.ascontiguousarray(x[b])} for b in range(B)]
    trace = bool(int(os.environ.get("KERNEL_TRACE", "0")))
    try:
        res = run_bass_kernel_spmd(_NC, in_maps, core_ids=list(range(B)), trace=trace)
    except Exception:
        if not trace:
            raise
        res = run_bass_kernel_spmd(_NC, in_maps, core_ids=list(range(B)), trace=False)
    LAST_RESULTS = res
    return np.stack([res.results[b]["out"] for b in range(B)], axis=0)


# revision 65
# speedup vs baseline: 1.0371x; 1.0371x over previous
"""Trainium2 Bass kernel for nn_Attention_43190191129190.

Model (per batch element b of 8):
    y   = x + dwconv3x3(x) + conv_b          (depthwise residual positional conv)
    qkv = y @ qkv_w.T ; split into q, k, v   (8 heads, dim 32)
    out = softmax(q k^T / sqrt(32)) v
    out = out @ out_w.T + out_b

Sharding: pure data-parallel, one batch element per NeuronCore (8 cores).

Per-core design (transposed [C, N] space; see kernel_baseline.py.bak for the
ancestor).  v2 changes vs the baseline:

  * Attention is split over the QUERY dim into two 512-token halves.  All 4
    head pairs run on half 0, whose normalization + out-projection + store
    overlap half 1's attention; only half 1's last pair remains in the tail.
  * PSUM re-plan: pst 2x[128,1024] (S^T double-buffer), ppv 1x[128,1024]
    (PV accumulator, freed by a single DVE evacuation), paux 1x[128,1024]
    (conv ct0 / v / qk tiles 1,3 / softmax-broadcast / projection).
  * Softmax normalization with no DRAM round trip: evacuate pv -> pc (SBUF),
    PE-broadcast the sums row (ones stationary), DVE reciprocal from PSUM,
    DVE multiply; deferred past the next pair's first S^T so PE never waits.
  * ScalarE runs exps only (one [128,1024] exp per pair/m covering both
    heads); all other evacuations are on DVE (preamble ones may use ScalarE
    while it is still idle).
  * Input DMAs spread over the four HWDGE queues + the Pool SWDGE path.
"""

import os

import numpy as np

import concourse.bass as bass
import concourse.tile as tile
from concourse import bacc, mybir
from concourse.bass_utils import run_bass_kernel_spmd

F32 = mybir.dt.float32
F32R = mybir.dt.float32r
AF = mybir.ActivationFunctionType

B, N, C = 8, 1024, 256
HEADS, DH = 8, 32
SCALE = DH ** -0.5
PAD = 34  # 32x32 spatial grid with 1-px halo
HALF = 512

TAPS = [(ky, kx) for ky in range(3) for kx in range(3)]
# Head pairs per query half.  Half 0 must order so q/k feature tiles 1,3
# (emitted during its pair 1) precede any head >= 4.  Half 1 re-pairs so the
# final pair (0,4) writes attn^T rows 0:32 of both chunks directly -- no
# repositioning DMA in the kernel tail.
PAIRS_H = [
    [(1, 3), (0, 2), (5, 7), (4, 6)],
    [(1, 3), (5, 7), (2, 6), (0, 4)],
]


def build_nc(debug_dump=False):
    nc = bacc.Bacc("TRN2", target_bir_lowering=False, debug=False, num_devices=8)

    x_d = nc.dram_tensor("x", (N, C), F32R, kind="ExternalInput").ap()
    qkvwT_d = nc.dram_tensor("qkv_wT", (C, 3 * C), F32R, kind="ExternalInput").ap()
    outwT_d = nc.dram_tensor("out_wT", (C, C), F32R, kind="ExternalInput").ap()
    taps_d = nc.dram_tensor("conv_taps", (128, 18), F32, kind="ExternalInput").ap()
    convb_d = nc.dram_tensor("conv_b_r", (1, C), F32R, kind="ExternalInput").ap()
    outb_d = nc.dram_tensor("out_b_r", (1, C), F32R, kind="ExternalInput").ap()
    id_d = nc.dram_tensor("id128", (128, 128), F32R, kind="ExternalInput").ap()
    out_d = nc.dram_tensor("out", (N, C), F32, kind="ExternalOutput").ap()
    dbg = {}
    if debug_dump:
        for name, shape in (
            ("d_yT", (128, 2, N)), ("d_qT", (128, 2, N)), ("d_kT", (128, 2, N)),
            ("d_v", (128, 8, 8 * 33)), ("d_attnT", (128, 2, N)),
        ):
            dbg[name] = nc.dram_tensor(name, shape, F32, kind="ExternalOutput").ap()

    with tile.TileContext(nc) as tc:
        with (
            tc.tile_pool(name="const", bufs=1) as const,
            tc.tile_pool(name="xin", bufs=1) as xin_p,
            tc.tile_pool(name="big", bufs=1) as big,
            tc.tile_pool(name="pT", bufs=8) as ppool,
            tc.tile_pool(name="rs", bufs=2) as rs_p,
            tc.tile_pool(name="tmp", bufs=2) as tmp_p,
            tc.tile_pool(name="tmp2", bufs=2) as tmp2_p,
            tc.tile_pool(name="outs", bufs=3) as outs_p,
            tc.tile_pool(name="pst", bufs=2, space="PSUM") as pst,
            tc.tile_pool(name="ppv", bufs=1, space="PSUM") as ppv,
            tc.tile_pool(name="paux", bufs=1, space="PSUM") as paux,
        ):
            # ---- input DMAs.  Per-DMA SEQ dispatch (~1.26us) serializing
            # per queue is the startup bound, so x comes in THREE combined
            # DMAs and the q/k/v weights follow on the same sync queue; the
            # conv diagonals are built on-device from a 9KB tap vector
            # (identity x per-partition scale) instead of a 1.2MB DMA.  The
            # small loads ride the parallel Pool SWDGE path.
            taps_sb = const.tile([128, 18], F32, tag="taps")
            nc.gpsimd.dma_start(taps_sb, taps_d)
            id_sb = const.tile([128, 128], F32R, tag="id")
            nc.sync.dma_start(id_sb, id_d)
            diag_sb = const.tile([128, 18, 128], F32R, tag="diag")
            xg = [None] * 3
            for g, (r0, ntile) in enumerate(((0, 2), (256, 3), (640, 3))):
                xg[g] = xin_p.tile([128, ntile, C], F32R, tag=f"xg{g}",
                                   name=f"xg{g}")
                nc.sync.dma_start(
                    xg[g],
                    x_d[r0:r0 + ntile * 128, :].rearrange(
                        "(t p) c -> p t c", p=128),
                )
            xins = [xg[0][:, 0, :], xg[0][:, 1, :],
                    xg[1][:, 0, :], xg[1][:, 1, :], xg[1][:, 2, :],
                    xg[2][:, 0, :], xg[2][:, 1, :], xg[2][:, 2, :]]
            qkvwT_sb = const.tile([128, 2, 3 * C], F32R, tag="qkvwT")
            qkvw_r = qkvwT_d.rearrange("(kc p) f -> p kc f", p=128)
            nc.sync.dma_start(qkvwT_sb[:, :, 0:512], qkvw_r[:, :, 0:512])
            nc.sync.dma_start(qkvwT_sb[:, :, 512:768], qkvw_r[:, :, 512:768])
            convb_sb = const.tile([1, C], F32R, tag="convb")
            nc.gpsimd.dma_start(convb_sb, convb_d)
            ones_sb = const.tile([1, N], F32R, tag="ones")
            nc.gpsimd.memset(ones_sb.bitcast(mybir.dt.uint32), 0x3F800000)
            outwT_sb = const.tile([128, 2, C], F32R, tag="outwT")
            nc.gpsimd.dma_start(outwT_sb, outwT_d.rearrange("(kc p) f -> p kc f", p=128))
            outb_sb = const.tile([1, C], F32R, tag="outb")
            nc.gpsimd.dma_start(outb_sb, outb_d)
            zerob_sb = const.tile([128, 1], F32, tag="zerob")
            nc.vector.memset(zerob_sb, 0.0)
            # dummy exp: hoists the ~1.3us exp_and_others ACT table load into
            # the idle startup window (the set also contains Copy, so the
            # preamble ScalarE copies share it)
            warm_sb = const.tile([1, 1], F32, tag="warm")
            nc.scalar.activation(
                warm_sb, zerob_sb[0:1, 0:1], AF.Exp,
                bias=zerob_sb[0:1], scale=1.0,
            )
            # conv diag ct0 built on ScalarE: diag[:, i, :] = id * tap_i[p]
            # (per-partition scale); ct1 is built on DVE once the transpose
            # evacuations are queued (see below)
            for i in range(9):
                nc.scalar.mul(diag_sb[:, i, :], id_sb, taps_sb[:, i:i + 1])
            # all-ones strip on every partition (PE broadcast stationary must
            # share its base partition with the moving operand)
            onesp_sb = const.tile([128, 32], F32R, tag="onesp")
            nc.gpsimd.memset(onesp_sb.bitcast(mybir.dt.uint32), 0x3F800000)

            # PE p-state warm-up: a stream of tiny matmuls keeps the tensor
            # engine continuously busy from ~0.8us so the 3us ramp completes
            # before the first real transpose (which then runs at full rate)
            pdum = pst.tile([128, 1024], F32, tag="ps", name="pdum")
            dum_sb = const.tile([1, 32], F32R, tag="dum")
            nc.vector.memset(dum_sb.bitcast(F32), 0.0)
            zb = zerob_sb.bitcast(F32R)
            for i in range(40):
                nc.tensor.matmul(
                    pdum[0:1, 0:32],
                    lhsT=zb[0:1, 0:1],
                    rhs=dum_sb,
                    start=(i == 0),
                    stop=(i == 39),
                )

            # ---- persistent activations ----
            xpadT = big.tile([128, 2, PAD * PAD], F32R, tag="xpadT")
            # zero only the 1-px halo ring (interior is fully overwritten)
            xpv = xpadT.bitcast(mybir.dt.uint32).rearrange(
                "p ct (h w) -> p ct h w", h=PAD
            )
            nc.gpsimd.memset(xpv[:, :, 0, :], 0)
            nc.gpsimd.memset(xpv[:, :, PAD - 1, :], 0)
            nc.gpsimd.memset(xpv[:, :, :, 0], 0)
            nc.gpsimd.memset(xpv[:, :, :, PAD - 1], 0)
            yT = big.tile([128, 2, N], F32R, tag="yT")
            qT = big.tile([128, 2, N], F32R, tag="qT")
            kT = big.tile([128, 2, N], F32R, tag="kT")
            vsb = big.tile([128, 8, 8 * 33], F32R, tag="v")
            # 1.0 everywhere (ones columns); v cols overwritten below
            nc.gpsimd.memset(vsb.bitcast(mybir.dt.uint32), 0x3F800000)
            attnT = big.tile([128, 2, N], F32R, tag="attnT")

            # preamble psum evacuations alternate between DVE and the (still
            # idle) ScalarE so neither queue gates slot turnover
            _cp = [0]

            def copy_alt(dst, src_ap):
                _cp[0] += 1
                if _cp[0] % 2:
                    nc.vector.tensor_copy(dst, src_ap)
                else:
                    nc.scalar.copy(dst, src_ap)

            # ---- transpose x into padded x^T, conv interleaved ----
            def emit_transpose(nt):
                tp = pst.tile([128, 1024], F32, tag="ps", name="tp")
                tpr = tp.bitcast(F32R)
                for ct in range(2):
                    nc.tensor.transpose(
                        tpr[:, 512 * ct: 512 * ct + 128],
                        xins[nt][:, 128 * ct: 128 * (ct + 1)],
                        id_sb,
                    )
                    dst = xpadT[:, ct, :].rearrange("p (h w) -> p h w", h=PAD)[
                        :, 1 + 4 * nt: 5 + 4 * nt, 1:33
                    ]
                    nc.vector.tensor_copy(
                        dst,
                        tp[:, 512 * ct: 512 * ct + 128].rearrange(
                            "p (a b) -> p a b", a=4
                        ),
                    )

            # conv accumulators: ct0 in the aux psum slot, ct1 in the (still
            # idle) PV slot, so transposes keep both pst slots
            cacc = [paux.tile([128, 1024], F32, tag="aux", name="cacc0"),
                    ppv.tile([128, 1024], F32, tag="pv", name="cacc1")]

            def emit_conv_part(ct, j, t0, t1, bias=False):
                cps = cacc[ct]
                view = xpadT[:, ct, :].rearrange("p (h w) -> p h w", h=PAD)
                for t in range(t0, t1):
                    ky, kx = TAPS[t]
                    nc.tensor.matmul(
                        cps[:, j * 512:(j + 1) * 512],
                        lhsT=diag_sb[:, ct * 9 + t, :],
                        rhs=view[:, ky + 16 * j: ky + 16 * j + 16, kx: kx + 32],
                        start=(t == 0),
                        stop=False,
                    )
                if bias:
                    nc.tensor.matmul(
                        cps[:, j * 512:(j + 1) * 512],
                        lhsT=convb_sb[0:1, 128 * ct: 128 * (ct + 1)],
                        rhs=ones_sb[0:1, j * 512:(j + 1) * 512],
                        start=False,
                        stop=True,
                    )

            def emit_conv_half(ct, j):
                emit_conv_part(ct, j, 0, 9, bias=True)

            # conv j=0 only needs padded rows 0..18 (x tiles 0..4) and
            # transposes 5..7 only feed conv j=1, so the preamble critical
            # path is transposes 0-4 -> conv(ct,0) -> q/k j=0.  conv j=1
            # (query tokens 512..1023, first needed at m-step 4 of pair 0)
            # is interleaved into pair 0's m-loop below.  The q/k j=0
            # contraction is split by feature chunk: the ct0 matmuls run
            # while conv ct1 still waits for its diag DMA.
            for nt in range(5):
                emit_transpose(nt)
            emit_conv_half(0, 0)
            for i in range(9, 18):
                nc.scalar.mul(diag_sb[:, i, :], id_sb, taps_sb[:, i:i + 1])
            nc.vector.tensor_copy(yT[:, 0, 0:512], cacc[0][:, 0:512])
            for nt in range(5, 8):
                emit_transpose(nt)
            qk_ps = {}
            for ft in (0, 2):
                qk_ps[ft] = pst.tile([128, 1024], F32, tag="ps", name="qkps")
                nc.tensor.matmul(
                    qk_ps[ft][:, 0:512],
                    lhsT=qkvwT_sb[:, 0, (0 if ft < 2 else 256): (0 if ft < 2 else 256) + 128],
                    rhs=yT[:, 0, 0:512],
                    start=True,
                    stop=False,
                )
            emit_conv_half(1, 0)
            nc.scalar.copy(yT[:, 1, 0:512], cacc[1][:, 0:512])
            for ft, dstT in ((0, qT), (2, kT)):
                nc.tensor.matmul(
                    qk_ps[ft][:, 0:512],
                    lhsT=qkvwT_sb[:, 1, (0 if ft < 2 else 256): (0 if ft < 2 else 256) + 128],
                    rhs=yT[:, 1, 0:512],
                    start=False,
                    stop=True,
                )
                nc.vector.tensor_copy(dstT[:, 0, 0:512], qk_ps[ft][:, 0:512])

            # ---- q^T / k^T feature tiles.  Tiles 0,2 (heads 0-3): the j=0
            # token halves run in the preamble, j=1 inside pair 0's m-loop.
            # Tiles 1,3 (heads 4-7) run inside pair 1's m-loop. ----
            def emit_qk(ft, pool, js=(0, 1), eng=None):
                dstT, dc = (qT, ft) if ft < 2 else (kT, ft - 2)
                fofs = 0 if ft < 2 else 256
                qps = pool.tile([128, 1024], F32, tag="ps" if pool is pst else "aux",
                                name="qps")
                for j in js:
                    for kc in range(2):
                        nc.tensor.matmul(
                            qps[:, j * 512:(j + 1) * 512],
                            lhsT=qkvwT_sb[:, kc, fofs + dc * 128: fofs + (dc + 1) * 128],
                            rhs=yT[:, kc, j * 512:(j + 1) * 512],
                            start=(kc == 0),
                            stop=(kc == 1),
                        )
                for j in js:
                    (eng or nc.vector).tensor_copy(
                        dstT[:, dc, j * 512:(j + 1) * 512],
                        qps[:, j * 512:(j + 1) * 512],
                    )

            def emit_v(nt):
                vps = paux.tile([128, 1024], F32, tag="aux", name="vps")
                for kc in range(2):
                    nc.tensor.matmul(
                        vps[:, 0:256],
                        lhsT=yT[:, kc, nt * 128:(nt + 1) * 128],
                        rhs=qkvwT_sb[:, kc, 512:768],
                        start=(kc == 0),
                        stop=(kc == 1),
                    )
                vv = vsb[:, nt, :].rearrange("p (hh c) -> p hh c", c=33)
                sv = vps[:, 0:256].rearrange("p (hh c) -> p hh c", c=32)
                nc.vector.tensor_copy(vv[:, :, 0:32], sv)  # [v_h | 1] per head

            emit_qk(0, pst, js=(0,))
            emit_qk(2, pst, js=(0,))

            # ---- out-projection of one 128-token tile (all 8 heads) ----
            # stores ride sync + the Pool SWDGE path: a scalar-queue store
            # would inject a ~1.3us DMA dispatch into the ACT SEQ mid-exp
            _oq = [nc.sync, nc.gpsimd, nc.sync, nc.gpsimd]

            def emit_proj(nt, pool=None):
                pool = pool or paux
                ops = pool.tile([128, 1024], F32,
                                tag="ps" if pool is pst else "aux", name="ops")
                for chunk in range(2):
                    nc.tensor.matmul(
                        ops[:, 0:256],
                        lhsT=attnT[:, chunk, nt * 128:(nt + 1) * 128],
                        rhs=outwT_sb[:, chunk, :],
                        start=(chunk == 0),
                        stop=False,
                    )
                nc.tensor.matmul(
                    ops[:, 0:256],
                    lhsT=ones_sb[0:1, 0:128],
                    rhs=outb_sb,
                    start=False,
                    stop=True,
                )
                osb = outs_p.tile([128, C], F32, tag="o")
                nc.vector.tensor_copy(osb, ops[:, 0:256])
                _oq[nt % 4].dma_start(out_d[nt * 128:(nt + 1) * 128, :], osb)

            # ---- attention: two query halves x four head pairs ----
            # aux work interleaved into the m-loops, a slice per m-step:
            #   (half0, pair0): conv j=1 + its yT evacs, q/k tile-0 j=1
            #                   slices, v(0..7) (v(m) must precede PV(m))
            #   (half0, pair1): qk tiles 1,3 (needed by pair 2 = heads 5,7)
            #   (half1, pair0/1): projection of half-0 tiles 0..3
            def pair_extra(half, ip, m):
                if half == 0 and ip == 0:
                    if m == 0:
                        emit_conv_part(0, 1, 0, 5)
                    elif m == 1:
                        emit_conv_part(0, 1, 5, 9, bias=True)
                        nc.vector.tensor_copy(yT[:, 0, 512:1024],
                                              cacc[0][:, 512:1024])
                    elif m == 2:
                        emit_conv_part(1, 1, 0, 5)
                        emit_v(0)
                    elif m == 3:
                        emit_conv_part(1, 1, 5, 9, bias=True)
                        nc.vector.tensor_copy(yT[:, 1, 512:1024],
                                              cacc[1][:, 512:1024])
                        emit_qk(2, paux, js=(1,))
                        emit_v(1)
                    elif m == 4:
                        emit_v(2)
                    elif m == 5:
                        emit_v(3)
                        emit_v(4)
                    elif m == 6:
                        emit_v(5)
                        emit_v(6)
                    elif m == 7:
                        emit_v(7)
                        emit_qk(0, paux, js=(1,))
                elif half == 0 and ip == 1:
                    # only the j=0 half of q tile 1 is needed before half 1;
                    # k tile 3 needs both halves by pair 2's m-step 4
                    if m == 3:
                        emit_qk(1, paux, js=(0,))
                    elif m == 5:
                        emit_qk(3, paux, js=(0,))
                    elif m == 6:
                        emit_qk(3, paux, js=(1,))
                elif half == 1 and ip == 0:
                    if m == 1:
                        emit_qk(1, paux, js=(1,))
                    elif m in (5, 7):
                        emit_proj((m - 5) // 2)
                elif half == 1 and ip == 1 and m in (1, 5):
                    emit_proj(2 + (m - 1) // 4)
                elif half == 1 and ip == 3 and m >= 4:
                    # partial projection of half-1 tiles: contributions from
                    # attn^T rows 32:64 and 96:128 (normalized well before
                    # this last pair -- rows 64:96 depend on pair (2,6)'s
                    # repositioning DMAs and join in the tail) plus the bias,
                    # staged to SBUF.
                    opsp = paux.tile([128, 1024], F32, tag="aux", name="opsp")
                    for chunk in range(2):
                        for a, k in ((32, 32), (64, 64)):
                            nc.tensor.matmul(
                                opsp[:, 0:256],
                                lhsT=attnT[a:a + k, chunk, m * 128:(m + 1) * 128],
                                rhs=outwT_sb[a:a + k, chunk, :],
                                start=(chunk == 0 and a == 32),
                                stop=False,
                                tile_position=(a, 0),
                            )
                    nc.tensor.matmul(
                        opsp[:, 0:256],
                        lhsT=ones_sb[0:1, 0:128],
                        rhs=outb_sb,
                        start=False,
                        stop=True,
                    )
                    if m < 7:
                        nc.vector.tensor_copy(partial4[:, m - 4, :],
                                              opsp[:, 0:256])
                    else:
                        opsp7[0] = opsp

            def emit_norm_b(half, hA, hB, pc, pool=None):
                # PE-broadcast the sums row (pc row 32) to 32 partitions,
                # reciprocal straight from psum, then normalize.
                pool = pool or paux
                bcp = pool.tile([128, 1024], F32,
                                tag="ps" if pool is pst else "aux", name="bcp")
                for j in range(2):
                    nc.tensor.matmul(
                        bcp[0:32, j * 512:(j + 1) * 512],
                        lhsT=onesp_sb[32:33, :],
                        rhs=pc[32:33, j * 512:(j + 1) * 512],
                        start=True,
                        stop=True,
                    )
                rs = rs_p.tile([128, 1024], F32, tag="rs")
                nc.vector.reciprocal(rs[0:32, :], bcp[0:32, :])
                pcb = pc.bitcast(F32)
                nofs = half * HALF
                for hd, h in ((0, hA), (1, hB)):
                    row = 32 * (h % 4)
                    hc = h // 4
                    cofs = hd * 512
                    if row == 0:
                        nc.vector.tensor_mul(
                            attnT[0:32, hc, nofs:nofs + HALF],
                            pcb[0:32, cofs:cofs + 512],
                            rs[0:32, cofs:cofs + 512],
                        )
                    else:
                        # reposition to the head's attn^T rows (DMA can shift
                        # partitions; DVE cannot)
                        pcs = tmp2_p.tile([128, 512], F32R, tag="pcs", name="pcs")
                        nc.vector.tensor_mul(
                            pcs[0:32, :],
                            pcb[0:32, cofs:cofs + 512],
                            rs[0:32, cofs:cofs + 512],
                        )
                        nc.sync.dma_start(
                            attnT[row:row + 32, hc, nofs:nofs + HALF], pcs[0:32, :]
                        )

            # The PV lag list is carried ACROSS pair boundaries: the trailing
            # PV accumulations of pair p are emitted during the first m-steps
            # of pair p+1, so the ScalarE exp stream never waits on a burst
            # of trailing PVs.  pv psum tiles are allocated lazily (at the
            # m==0 PV) so pair 0's conv-j1 accumulators can share the slots;
            # pair 0 uses lag 3 since v(0) only exists from its m-step 2.
            # The pv evacuation (one DVE copy) follows the m==7 PV; the rest
            # of the normalization lands >= m-step 3 of the next pair, well
            # past any PE dependency.
            pend = []  # (pid, half, hA, hB, m, pT) awaiting PV matmuls
            pending_norm = None  # (half, hA, hB, pc)
            pvt = {}  # pid -> lazily allocated pv psum tile
            partial4 = big.tile([128, 4, C], F32R, tag="partial4")
            opsp7 = [None]  # last partial-proj psum, evacuated in the tail

            def emit_pv(pid, half, hA, hB, m, pT):
                nonlocal pending_norm
                if m == 0:
                    pvt[pid] = ppv.tile([128, 1024], F32, tag="pv", name="pv")
                pv = pvt[pid]
                for hd, h in ((0, hA), (1, hB)):
                    nc.tensor.matmul(
                        pv[0:33, hd * 512:(hd + 1) * 512],
                        lhsT=vsb[:, m, 33 * h: 33 * h + 33],
                        rhs=pT[:, hd * 512:(hd + 1) * 512],
                        start=(m == 0),
                        stop=(m == 7),
                    )
                if m == 7:
                    pc = tmp_p.tile([128, 1024], F32R, tag="pc", name="pc")
                    nc.vector.tensor_copy(pc[0:33, :], pv[0:33, :])
                    pending_norm = (half, hA, hB, pc)
                    del pvt[pid]

            for half in range(2):
                for ip, (hA, hB) in enumerate(PAIRS_H[half]):
                    pid = half * 4 + ip
                    lag = 3 if pid == 0 else 2
                    for m in range(8):
                        st = pst.tile([128, 1024], F32, tag="ps")
                        for hd, h in ((0, hA), (1, hB)):
                            a = 32 * (h % 4)
                            hc = h // 4
                            nc.tensor.matmul(
                                st[:, hd * 512:(hd + 1) * 512],
                                lhsT=kT[a:a + 32, hc, m * 128:(m + 1) * 128],
                                rhs=qT[a:a + 32, hc, half * HALF: half * HALF + 512],
                                start=True,
                                stop=True,
                                tile_position=(a, 0),
                            )
                        # drain up to two prev-pair PVs per step (so the pv
                        # evacuation lands >= 2 steps before this pair's m==0
                        # PV reuses the psum slot), plus one own-pair PV once
                        # past the lag
                        for _ in range(2):
                            if pend and pend[0][0] != pid:
                                emit_pv(*pend.pop(0))
                        if pend and pend[0][0] == pid and len(pend) > lag:
                            emit_pv(*pend.pop(0))
                        if pending_norm is not None and m >= 2:
                            emit_norm_b(*pending_norm)
                            pending_norm = None
                        pT = ppool.tile([128, 1024], F32R, tag="pT")
                        nc.scalar.activation(pT, st, AF.Exp, bias=zerob_sb, scale=SCALE)
                        pair_extra(half, ip, m)
                        pend.append((pid, half, hA, hB, m, pT))
            # ---- tail: drain the PV backlog; the last pair's (m==7) PVs are
            # followed per-head by a short normalize chain (copy, broadcast,
            # reciprocal, multiply -- heads 0 and 4 both land on rows 0:32 so
            # there is no repositioning), then the half-1 projection finishes
            # with the two K=32 row-0 contributions per tile and stores.
            while len(pend) > 1:
                emit_pv(*pend.pop(0))
            lpid, lhalf, lhA, lhB, lm, lpT = pend.pop(0)
            pv = pvt[lpid]
            nofs = lhalf * HALF
            for hd, h in ((0, lhA), (1, lhB)):
                nc.tensor.matmul(
                    pv[0:33, hd * 512:(hd + 1) * 512],
                    lhsT=vsb[:, lm, 33 * h: 33 * h + 33],
                    rhs=lpT[:, hd * 512:(hd + 1) * 512],
                    start=False,
                    stop=True,
                )
            # evacuate the two heads' unnormalized tiles + sums, one on the
            # (now idle) ScalarE and one on DVE so they overlap; the deferred
            # m==7 partial-proj evacuation is emitted AFTER the reciprocal so
            # the normalize chain jumps ahead of it in the DVE queue.
            pc2 = tmp_p.tile([128, 1024], F32R, tag="pc", name="pc2")
            nc.scalar.copy(pc2[0:33, 0:512], pv[0:33, 0:512])
            nc.scalar.copy(pc2[0:33, 512:1024], pv[0:33, 512:1024])
            # broadcast both heads' sums, ONE reciprocal over the combined
            # rows, then normalize (both heads land on rows 0:32)
            bcp2 = pst.tile([128, 1024], F32, tag="ps", name="bcp2")
            for hd in range(2):
                nc.tensor.matmul(
                    bcp2[0:32, hd * 512:(hd + 1) * 512],
                    lhsT=onesp_sb[32:33, :],
                    rhs=pc2[32:33, hd * 512:(hd + 1) * 512],
                    start=True,
                    stop=True,
                )
            rs2t = rs_p.tile([128, 1024], F32, tag="rs", name="rs2t")
            nc.vector.reciprocal(rs2t[0:32, :], bcp2[0:32, :])
            nc.vector.tensor_copy(partial4[:, 3, :], opsp7[0][:, 0:256])
            pc2b = pc2.bitcast(F32)
            for hd, h in ((0, lhA), (1, lhB)):
                nc.vector.tensor_mul(
                    attnT[0:32, h // 4, nofs:nofs + HALF],
                    pc2b[0:32, hd * 512:(hd + 1) * 512],
                    rs2t[0:32, hd * 512:(hd + 1) * 512],
                )

            if debug_dump:
                nc.sync.dma_start(dbg["d_yT"], yT.bitcast(F32))
                nc.sync.dma_start(dbg["d_qT"], qT.bitcast(F32))
                nc.sync.dma_start(dbg["d_kT"], kT.bitcast(F32))
                nc.sync.dma_start(dbg["d_v"], vsb.bitcast(F32))
                nc.sync.dma_start(dbg["d_attnT"], attnT.bitcast(F32))

            # per psum tile (two output tiles each, separate banks): fold
            # the staged partial via an identity matmul, add the row-0
            # contributions, evacuate + store per half so the A store
            # launches while B's matmuls still run
            opsf = [pst.tile([128, 1024], F32, tag="ps", name="opsfA"),
                    paux.tile([128, 1024], F32, tag="aux", name="opsfB")]
            for half_t, q in ((0, nc.sync), (1, nc.scalar)):
                for i in (0, 1):
                    nt = 4 + half_t * 2 + i
                    sl = opsf[half_t][:, i * 512:i * 512 + 256]
                    nc.tensor.matmul(
                        sl,
                        lhsT=id_sb,
                        rhs=partial4[:, half_t * 2 + i, :],
                        start=True,
                        stop=False,
                    )
                    for chunk in range(2):
                        nc.tensor.matmul(
                            sl,
                            lhsT=attnT[0:32, chunk, nt * 128:(nt + 1) * 128],
                            rhs=outwT_sb[0:32, chunk, :],
                            start=False,
                            stop=(chunk == 1),
                            tile_position=(0, 0),
                        )
                osb2 = outs_p.tile([128, 2, C], F32, tag="o", name="osb2")
                osrc = opsf[half_t].rearrange("p (t c) -> p t c", t=2)[:, :, 0:256]
                if half_t == 0:
                    nc.vector.tensor_copy(osb2, osrc)
                else:
                    nc.scalar.copy(osb2, osrc)
                r0 = 512 + half_t * 256
                q.dma_start(
                    out_d[r0:r0 + 256, :].rearrange("(t p) c -> p t c", p=128),
                    osb2,
                )

    nc.compile()
    return nc


_NC = None
LAST_RESULTS = None


def _host_prep(conv_w, conv_b, qkv_w, out_w, out_b):
    conv_w = np.asarray(conv_w, np.float32).reshape(C, 3, 3)
    taps = np.zeros((128, 18), np.float32)
    for ct in range(2):
        for t, (ky, kx) in enumerate(TAPS):
            d = conv_w[128 * ct: 128 * (ct + 1), ky, kx].copy()
            if (ky, kx) == (1, 1):
                d += 1.0  # residual connection folded into the center tap
            taps[:, ct * 9 + t] = d
    return {
        "qkv_wT": np.ascontiguousarray(np.asarray(qkv_w, np.float32).T),
        "out_wT": np.ascontiguousarray(np.asarray(out_w, np.float32).T),
        "conv_taps": taps,
        "conv_b_r": np.asarray(conv_b, np.float32).reshape(1, C),
        "out_b_r": np.asarray(out_b, np.float32).reshape(1, C),
        "id128": np.eye(128, dtype=np.float32),
    }


def kernel(x, conv_w, conv_b, qkv_w, out_w, out_b):
    global _NC, LAST_RESULTS
    if _NC is None:
        _NC = build_nc()
    x = np.asarray(x, np.float32)
    shared = _host_prep(conv_w, conv_b, qkv_w, out_w, out_b)
    in_maps = [{**shared, "x": np.ascontiguousarray(x[b])} for b in range(B)]
    trace = bool(int(os.environ.get("KERNEL_TRACE", "0")))
    try:
        res = run_bass_kernel_spmd(_NC, in_maps, core_ids=list(range(B)), trace=trace)
    except Exception:
        if not trace:
            raise
        res = run_bass_kernel_spmd(_NC, in_maps, core_ids=list(range(B)), trace=False)
    LAST_RESULTS = res
    return np.stack([res.results[b]["out"] for b in range(B)], axis=0)


# revision 67
# speedup vs baseline: 1.0600x; 1.0221x over previous
"""Trainium2 Bass kernel for nn_Attention_43190191129190.

Model (per batch element b of 8):
    y   = x + dwconv3x3(x) + conv_b          (depthwise residual positional conv)
    qkv = y @ qkv_w.T ; split into q, k, v   (8 heads, dim 32)
    out = softmax(q k^T / sqrt(32)) v
    out = out @ out_w.T + out_b

Sharding: pure data-parallel, one batch element per NeuronCore (8 cores).

Per-core design (transposed [C, N] space; see kernel_baseline.py.bak for the
ancestor).  v2 changes vs the baseline:

  * Attention is split over the QUERY dim into two 512-token halves.  All 4
    head pairs run on half 0, whose normalization + out-projection + store
    overlap half 1's attention; only half 1's last pair remains in the tail.
  * PSUM re-plan: pst 2x[128,1024] (S^T double-buffer), ppv 1x[128,1024]
    (PV accumulator, freed by a single DVE evacuation), paux 1x[128,1024]
    (conv ct0 / v / qk tiles 1,3 / softmax-broadcast / projection).
  * Softmax normalization with no DRAM round trip: evacuate pv -> pc (SBUF),
    PE-broadcast the sums row (ones stationary), DVE reciprocal from PSUM,
    DVE multiply; deferred past the next pair's first S^T so PE never waits.
  * ScalarE runs exps only (one [128,1024] exp per pair/m covering both
    heads); all other evacuations are on DVE (preamble ones may use ScalarE
    while it is still idle).
  * Input DMAs spread over the four HWDGE queues + the Pool SWDGE path.
"""

import os

import numpy as np

import concourse.bass as bass
import concourse.tile as tile
from concourse import bacc, mybir
from concourse.bass_utils import run_bass_kernel_spmd

F32 = mybir.dt.float32
F32R = mybir.dt.float32r
AF = mybir.ActivationFunctionType

B, N, C = 8, 1024, 256
HEADS, DH = 8, 32
SCALE = DH ** -0.5
PAD = 34  # 32x32 spatial grid with 1-px halo
HALF = 512

TAPS = [(ky, kx) for ky in range(3) for kx in range(3)]
# Head pairs per query half.  Half 0 must order so q/k feature tiles 1,3
# (emitted during its pair 1) precede any head >= 4.  Half 1 re-pairs so the
# final pair (0,4) writes attn^T rows 0:32 of both chunks directly -- no
# repositioning DMA in the kernel tail.
PAIRS_H = [
    [(1, 3), (0, 2), (5, 7), (4, 6)],
    [(1, 3), (5, 7), (2, 6), (0, 4)],
]


def build_nc(debug_dump=False):
    nc = bacc.Bacc("TRN2", target_bir_lowering=False, debug=False, num_devices=8)

    x_d = nc.dram_tensor("x", (N, C), F32R, kind="ExternalInput").ap()
    qkvwT_d = nc.dram_tensor("qkv_wT", (C, 3 * C), F32R, kind="ExternalInput").ap()
    outwT_d = nc.dram_tensor("out_wT", (C, C), F32R, kind="ExternalInput").ap()
    taps_d = nc.dram_tensor("conv_taps", (128, 18), F32, kind="ExternalInput").ap()
    convb_d = nc.dram_tensor("conv_b_r", (1, C), F32R, kind="ExternalInput").ap()
    outb_d = nc.dram_tensor("out_b_r", (1, C), F32R, kind="ExternalInput").ap()
    id_d = nc.dram_tensor("id128", (128, 128), F32R, kind="ExternalInput").ap()
    out_d = nc.dram_tensor("out", (N, C), F32, kind="ExternalOutput").ap()
    dbg = {}
    if debug_dump:
        for name, shape in (
            ("d_yT", (128, 2, N)), ("d_qT", (128, 2, N)), ("d_kT", (128, 2, N)),
            ("d_v", (128, 8, 8 * 33)), ("d_attnT", (128, 2, N)),
        ):
            dbg[name] = nc.dram_tensor(name, shape, F32, kind="ExternalOutput").ap()

    with tile.TileContext(nc) as tc:
        with (
            tc.tile_pool(name="const", bufs=1) as const,
            tc.tile_pool(name="xin", bufs=1) as xin_p,
            tc.tile_pool(name="big", bufs=1) as big,
            tc.tile_pool(name="pT", bufs=8) as ppool,
            tc.tile_pool(name="rs", bufs=2) as rs_p,
            tc.tile_pool(name="tmp", bufs=2) as tmp_p,
            tc.tile_pool(name="tmp2", bufs=2) as tmp2_p,
            tc.tile_pool(name="outs", bufs=3) as outs_p,
            tc.tile_pool(name="pst", bufs=2, space="PSUM") as pst,
            tc.tile_pool(name="ppv", bufs=1, space="PSUM") as ppv,
            tc.tile_pool(name="paux", bufs=1, space="PSUM") as paux,
        ):
            # ---- input DMAs.  Per-DMA SEQ dispatch (~1.26us) serializing
            # per queue is the startup bound, so x comes in THREE combined
            # DMAs and the q/k/v weights follow on the same sync queue; the
            # conv diagonals are built on-device from a 9KB tap vector
            # (identity x per-partition scale) instead of a 1.2MB DMA.  The
            # small loads ride the parallel Pool SWDGE path.
            taps_sb = const.tile([128, 18], F32, tag="taps")
            nc.gpsimd.dma_start(taps_sb, taps_d)
            id_sb = const.tile([128, 128], F32R, tag="id")
            nc.sync.dma_start(id_sb, id_d)
            diag_sb = const.tile([128, 18, 128], F32R, tag="diag")
            xg = [None] * 3
            for g, (r0, ntile) in enumerate(((0, 2), (256, 3), (640, 3))):
                xg[g] = xin_p.tile([128, ntile, C], F32R, tag=f"xg{g}",
                                   name=f"xg{g}")
                nc.sync.dma_start(
                    xg[g],
                    x_d[r0:r0 + ntile * 128, :].rearrange(
                        "(t p) c -> p t c", p=128),
                )
            xins = [xg[0][:, 0, :], xg[0][:, 1, :],
                    xg[1][:, 0, :], xg[1][:, 1, :], xg[1][:, 2, :],
                    xg[2][:, 0, :], xg[2][:, 1, :], xg[2][:, 2, :]]
            qkvwT_sb = const.tile([128, 2, 3 * C], F32R, tag="qkvwT")
            qkvw_r = qkvwT_d.rearrange("(kc p) f -> p kc f", p=128)
            nc.sync.dma_start(qkvwT_sb[:, :, 0:512], qkvw_r[:, :, 0:512])
            nc.sync.dma_start(qkvwT_sb[:, :, 512:768], qkvw_r[:, :, 512:768])
            convb_sb = const.tile([1, C], F32R, tag="convb")
            nc.gpsimd.dma_start(convb_sb, convb_d)
            ones_sb = const.tile([1, N], F32R, tag="ones")
            nc.gpsimd.memset(ones_sb.bitcast(mybir.dt.uint32), 0x3F800000)
            outwT_sb = const.tile([128, 2, C], F32R, tag="outwT")
            nc.gpsimd.dma_start(outwT_sb, outwT_d.rearrange("(kc p) f -> p kc f", p=128))
            outb_sb = const.tile([1, C], F32R, tag="outb")
            nc.gpsimd.dma_start(outb_sb, outb_d)
            zerob_sb = const.tile([128, 1], F32, tag="zerob")
            nc.vector.memset(zerob_sb, 0.0)
            # dummy exp: hoists the ~1.3us exp_and_others ACT table load into
            # the idle startup window (the set also contains Copy, so the
            # preamble ScalarE copies share it)
            warm_sb = const.tile([1, 1], F32, tag="warm")
            nc.scalar.activation(
                warm_sb, zerob_sb[0:1, 0:1], AF.Exp,
                bias=zerob_sb[0:1], scale=1.0,
            )
            # conv diag ct0 built on ScalarE: diag[:, i, :] = id * tap_i[p]
            # (per-partition scale); ct1 is built on DVE once the transpose
            # evacuations are queued (see below)
            for i in range(9):
                nc.scalar.mul(diag_sb[:, i, :], id_sb, taps_sb[:, i:i + 1])
            # all-ones strip on every partition (PE broadcast stationary must
            # share its base partition with the moving operand)
            onesp_sb = const.tile([128, 32], F32R, tag="onesp")
            nc.gpsimd.memset(onesp_sb.bitcast(mybir.dt.uint32), 0x3F800000)

            # PE p-state warm-up: a stream of tiny matmuls keeps the tensor
            # engine continuously busy from ~0.8us so the 3us ramp completes
            # before the first real transpose (which then runs at full rate)
            pdum = pst.tile([128, 1024], F32, tag="ps", name="pdum")
            dum_sb = const.tile([1, 32], F32R, tag="dum")
            nc.vector.memset(dum_sb.bitcast(F32), 0.0)
            zb = zerob_sb.bitcast(F32R)
            for i in range(40):
                nc.tensor.matmul(
                    pdum[0:1, 0:32],
                    lhsT=zb[0:1, 0:1],
                    rhs=dum_sb,
                    start=(i == 0),
                    stop=(i == 39),
                )

            # ---- persistent activations ----
            xpadT = big.tile([128, 2, PAD * PAD], F32R, tag="xpadT")
            # zero only the 1-px halo ring (interior is fully overwritten)
            xpv = xpadT.bitcast(mybir.dt.uint32).rearrange(
                "p ct (h w) -> p ct h w", h=PAD
            )
            # halo memsets on DVE: the Pool queue is busy dispatching SWDGE
            # loads until ~4.5us, which would gate the first conv matmul
            nc.vector.memset(xpv[:, :, 0, :], 0)
            nc.vector.memset(xpv[:, :, PAD - 1, :], 0)
            nc.vector.memset(xpv[:, :, :, 0], 0)
            nc.vector.memset(xpv[:, :, :, PAD - 1], 0)
            yT = big.tile([128, 2, N], F32R, tag="yT")
            qT = big.tile([128, 2, N], F32R, tag="qT")
            kT = big.tile([128, 2, N], F32R, tag="kT")
            vsb = big.tile([128, 8, 8 * 33], F32R, tag="v")
            # 1.0 everywhere (ones columns); v cols overwritten below
            nc.gpsimd.memset(vsb.bitcast(mybir.dt.uint32), 0x3F800000)
            attnT = big.tile([128, 2, N], F32R, tag="attnT")

            # preamble psum evacuations alternate between DVE and the (still
            # idle) ScalarE so neither queue gates slot turnover
            _cp = [0]

            def copy_alt(dst, src_ap):
                _cp[0] += 1
                if _cp[0] % 2:
                    nc.vector.tensor_copy(dst, src_ap)
                else:
                    nc.scalar.copy(dst, src_ap)

            # ---- transpose x into padded x^T, conv interleaved ----
            def emit_transpose(nt):
                tp = pst.tile([128, 1024], F32, tag="ps", name="tp")
                tpr = tp.bitcast(F32R)
                for ct in range(2):
                    nc.tensor.transpose(
                        tpr[:, 512 * ct: 512 * ct + 128],
                        xins[nt][:, 128 * ct: 128 * (ct + 1)],
                        id_sb,
                    )
                # both channel tiles evacuated in ONE 4D-AP copy (halves the
                # per-instruction overhead on the DVE chain that paces conv)
                dst = xpadT.rearrange("p ct (h w) -> p ct h w", h=PAD)[
                    :, :, 1 + 4 * nt: 5 + 4 * nt, 1:33
                ]
                nc.vector.tensor_copy(
                    dst,
                    tp.rearrange("p (ct w) -> p ct w", ct=2)[:, :, 0:128]
                    .rearrange("p ct (a b) -> p ct a b", a=4),
                )

            # conv accumulators: ct0 in the aux psum slot, ct1 in the (still
            # idle) PV slot, so transposes keep both pst slots
            cacc = [paux.tile([128, 1024], F32, tag="aux", name="cacc0"),
                    ppv.tile([128, 1024], F32, tag="pv", name="cacc1")]

            def emit_conv_part(ct, j, t0, t1, bias=False):
                cps = cacc[ct]
                view = xpadT[:, ct, :].rearrange("p (h w) -> p h w", h=PAD)
                for t in range(t0, t1):
                    ky, kx = TAPS[t]
                    nc.tensor.matmul(
                        cps[:, j * 512:(j + 1) * 512],
                        lhsT=diag_sb[:, ct * 9 + t, :],
                        rhs=view[:, ky + 16 * j: ky + 16 * j + 16, kx: kx + 32],
                        start=(t == 0),
                        stop=False,
                    )
                if bias:
                    nc.tensor.matmul(
                        cps[:, j * 512:(j + 1) * 512],
                        lhsT=convb_sb[0:1, 128 * ct: 128 * (ct + 1)],
                        rhs=ones_sb[0:1, j * 512:(j + 1) * 512],
                        start=False,
                        stop=True,
                    )

            def emit_conv_half(ct, j):
                emit_conv_part(ct, j, 0, 9, bias=True)

            # conv j=0 only needs padded rows 0..18 (x tiles 0..4) and
            # transposes 5..7 only feed conv j=1, so the preamble critical
            # path is transposes 0-4 -> conv(ct,0) -> q/k j=0.  conv j=1
            # (query tokens 512..1023, first needed at m-step 4 of pair 0)
            # is interleaved into pair 0's m-loop below.  The q/k j=0
            # contraction is split by feature chunk: the ct0 matmuls run
            # while conv ct1 still waits for its diag DMA.
            for nt in range(5):
                emit_transpose(nt)
            emit_conv_half(0, 0)
            for i in range(9, 18):
                nc.scalar.mul(diag_sb[:, i, :], id_sb, taps_sb[:, i:i + 1])
            nc.vector.tensor_copy(yT[:, 0, 0:512], cacc[0][:, 0:512])
            for nt in range(5, 8):
                emit_transpose(nt)
            qk_ps = {}
            for ft in (0, 2):
                qk_ps[ft] = pst.tile([128, 1024], F32, tag="ps", name="qkps")
                nc.tensor.matmul(
                    qk_ps[ft][:, 0:512],
                    lhsT=qkvwT_sb[:, 0, (0 if ft < 2 else 256): (0 if ft < 2 else 256) + 128],
                    rhs=yT[:, 0, 0:512],
                    start=True,
                    stop=False,
                )
            emit_conv_half(1, 0)
            nc.scalar.copy(yT[:, 1, 0:512], cacc[1][:, 0:512])
            for ft, dstT in ((0, qT), (2, kT)):
                nc.tensor.matmul(
                    qk_ps[ft][:, 0:512],
                    lhsT=qkvwT_sb[:, 1, (0 if ft < 2 else 256): (0 if ft < 2 else 256) + 128],
                    rhs=yT[:, 1, 0:512],
                    start=False,
                    stop=True,
                )
                nc.vector.tensor_copy(dstT[:, 0, 0:512], qk_ps[ft][:, 0:512])

            # ---- q^T / k^T feature tiles.  Tiles 0,2 (heads 0-3): the j=0
            # token halves run in the preamble, j=1 inside pair 0's m-loop.
            # Tiles 1,3 (heads 4-7) run inside pair 1's m-loop. ----
            def emit_qk(ft, pool, js=(0, 1), eng=None):
                dstT, dc = (qT, ft) if ft < 2 else (kT, ft - 2)
                fofs = 0 if ft < 2 else 256
                qps = pool.tile([128, 1024], F32, tag="ps" if pool is pst else "aux",
                                name="qps")
                for j in js:
                    for kc in range(2):
                        nc.tensor.matmul(
                            qps[:, j * 512:(j + 1) * 512],
                            lhsT=qkvwT_sb[:, kc, fofs + dc * 128: fofs + (dc + 1) * 128],
                            rhs=yT[:, kc, j * 512:(j + 1) * 512],
                            start=(kc == 0),
                            stop=(kc == 1),
                        )
                for j in js:
                    (eng or nc.vector).tensor_copy(
                        dstT[:, dc, j * 512:(j + 1) * 512],
                        qps[:, j * 512:(j + 1) * 512],
                    )

            def emit_v(nt):
                vps = paux.tile([128, 1024], F32, tag="aux", name="vps")
                for kc in range(2):
                    nc.tensor.matmul(
                        vps[:, 0:256],
                        lhsT=yT[:, kc, nt * 128:(nt + 1) * 128],
                        rhs=qkvwT_sb[:, kc, 512:768],
                        start=(kc == 0),
                        stop=(kc == 1),
                    )
                vv = vsb[:, nt, :].rearrange("p (hh c) -> p hh c", c=33)
                sv = vps[:, 0:256].rearrange("p (hh c) -> p hh c", c=32)
                nc.vector.tensor_copy(vv[:, :, 0:32], sv)  # [v_h | 1] per head

            emit_qk(0, pst, js=(0,))
            emit_qk(2, pst, js=(0,))

            # ---- out-projection of one 128-token tile (all 8 heads) ----
            # stores ride sync + the Pool SWDGE path: a scalar-queue store
            # would inject a ~1.3us DMA dispatch into the ACT SEQ mid-exp
            _oq = [nc.sync, nc.gpsimd, nc.sync, nc.gpsimd]

            def emit_proj(nt, pool=None):
                pool = pool or paux
                ops = pool.tile([128, 1024], F32,
                                tag="ps" if pool is pst else "aux", name="ops")
                for chunk in range(2):
                    nc.tensor.matmul(
                        ops[:, 0:256],
                        lhsT=attnT[:, chunk, nt * 128:(nt + 1) * 128],
                        rhs=outwT_sb[:, chunk, :],
                        start=(chunk == 0),
                        stop=False,
                    )
                nc.tensor.matmul(
                    ops[:, 0:256],
                    lhsT=ones_sb[0:1, 0:128],
                    rhs=outb_sb,
                    start=False,
                    stop=True,
                )
                osb = outs_p.tile([128, C], F32, tag="o")
                nc.vector.tensor_copy(osb, ops[:, 0:256])
                _oq[nt % 4].dma_start(out_d[nt * 128:(nt + 1) * 128, :], osb)

            # ---- attention: two query halves x four head pairs ----
            # aux work interleaved into the m-loops, a slice per m-step:
            #   (half0, pair0): conv j=1 + its yT evacs, q/k tile-0 j=1
            #                   slices, v(0..7) (v(m) must precede PV(m))
            #   (half0, pair1): qk tiles 1,3 (needed by pair 2 = heads 5,7)
            #   (half1, pair0/1): projection of half-0 tiles 0..3
            def pair_extra(half, ip, m):
                if half == 0 and ip == 0:
                    if m == 0:
                        emit_conv_part(0, 1, 0, 5)
                    elif m == 1:
                        emit_conv_part(0, 1, 5, 9, bias=True)
                        nc.vector.tensor_copy(yT[:, 0, 512:1024],
                                              cacc[0][:, 512:1024])
                    elif m == 2:
                        emit_conv_part(1, 1, 0, 5)
                        emit_v(0)
                    elif m == 3:
                        emit_conv_part(1, 1, 5, 9, bias=True)
                        nc.vector.tensor_copy(yT[:, 1, 512:1024],
                                              cacc[1][:, 512:1024])
                        emit_qk(2, paux, js=(1,))
                        emit_v(1)
                    elif m == 4:
                        emit_v(2)
                    elif m == 5:
                        emit_v(3)
                        emit_v(4)
                    elif m == 6:
                        emit_v(5)
                        emit_v(6)
                    elif m == 7:
                        emit_v(7)
                        emit_qk(0, paux, js=(1,))
                elif half == 0 and ip == 1:
                    # only the j=0 half of q tile 1 is needed before half 1;
                    # k tile 3 needs both halves by pair 2's m-step 4
                    if m == 3:
                        emit_qk(1, paux, js=(0,))
                    elif m == 5:
                        emit_qk(3, paux, js=(0,))
                    elif m == 6:
                        emit_qk(3, paux, js=(1,))
                elif half == 1 and ip == 0:
                    if m == 1:
                        emit_qk(1, paux, js=(1,))
                    elif m in (5, 7):
                        emit_proj((m - 5) // 2)
                elif half == 1 and ip == 1 and m in (1, 5):
                    emit_proj(2 + (m - 1) // 4)
                elif half == 1 and ip == 3 and m >= 4:
                    # partial projection of half-1 tiles: contributions from
                    # attn^T rows 32:64 and 96:128 (normalized well before
                    # this last pair -- rows 64:96 depend on pair (2,6)'s
                    # repositioning DMAs and join in the tail) plus the bias,
                    # staged to SBUF.
                    opsp = paux.tile([128, 1024], F32, tag="aux", name="opsp")
                    for chunk in range(2):
                        for a, k in ((32, 32), (64, 64)):
                            nc.tensor.matmul(
                                opsp[:, 0:256],
                                lhsT=attnT[a:a + k, chunk, m * 128:(m + 1) * 128],
                                rhs=outwT_sb[a:a + k, chunk, :],
                                start=(chunk == 0 and a == 32),
                                stop=False,
                                tile_position=(a, 0),
                            )
                    nc.tensor.matmul(
                        opsp[:, 0:256],
                        lhsT=ones_sb[0:1, 0:128],
                        rhs=outb_sb,
                        start=False,
                        stop=True,
                    )
                    if m < 7:
                        nc.vector.tensor_copy(partial4[:, m - 4, :],
                                              opsp[:, 0:256])
                    else:
                        opsp7[0] = opsp

            def emit_norm_b(half, hA, hB, pc, pool=None):
                # PE-broadcast the sums row (pc row 32) to 32 partitions,
                # reciprocal straight from psum, then normalize.
                pool = pool or paux
                bcp = pool.tile([128, 1024], F32,
                                tag="ps" if pool is pst else "aux", name="bcp")
                for j in range(2):
                    nc.tensor.matmul(
                        bcp[0:32, j * 512:(j + 1) * 512],
                        lhsT=onesp_sb[32:33, :],
                        rhs=pc[32:33, j * 512:(j + 1) * 512],
                        start=True,
                        stop=True,
                    )
                rs = rs_p.tile([128, 1024], F32, tag="rs")
                nc.vector.reciprocal(rs[0:32, :], bcp[0:32, :])
                pcb = pc.bitcast(F32)
                nofs = half * HALF
                for hd, h in ((0, hA), (1, hB)):
                    row = 32 * (h % 4)
                    hc = h // 4
                    cofs = hd * 512
                    if row == 0:
                        nc.vector.tensor_mul(
                            attnT[0:32, hc, nofs:nofs + HALF],
                            pcb[0:32, cofs:cofs + 512],
                            rs[0:32, cofs:cofs + 512],
                        )
                    else:
                        # reposition to the head's attn^T rows (DMA can shift
                        # partitions; DVE cannot)
                        pcs = tmp2_p.tile([128, 512], F32R, tag="pcs", name="pcs")
                        nc.vector.tensor_mul(
                            pcs[0:32, :],
                            pcb[0:32, cofs:cofs + 512],
                            rs[0:32, cofs:cofs + 512],
                        )
                        nc.sync.dma_start(
                            attnT[row:row + 32, hc, nofs:nofs + HALF], pcs[0:32, :]
                        )

            # The PV lag list is carried ACROSS pair boundaries: the trailing
            # PV accumulations of pair p are emitted during the first m-steps
            # of pair p+1, so the ScalarE exp stream never waits on a burst
            # of trailing PVs.  pv psum tiles are allocated lazily (at the
            # m==0 PV) so pair 0's conv-j1 accumulators can share the slots;
            # pair 0 uses lag 3 since v(0) only exists from its m-step 2.
            # The pv evacuation (one DVE copy) follows the m==7 PV; the rest
            # of the normalization lands >= m-step 3 of the next pair, well
            # past any PE dependency.
            pend = []  # (pid, half, hA, hB, m, pT) awaiting PV matmuls
            pending_norm = None  # (half, hA, hB, pc)
            pvt = {}  # pid -> lazily allocated pv psum tile
            partial4 = big.tile([128, 4, C], F32R, tag="partial4")
            opsp7 = [None]  # last partial-proj psum, evacuated in the tail

            def emit_pv(pid, half, hA, hB, m, pT):
                nonlocal pending_norm
                if m == 0:
                    pvt[pid] = ppv.tile([128, 1024], F32, tag="pv", name="pv")
                pv = pvt[pid]
                for hd, h in ((0, hA), (1, hB)):
                    nc.tensor.matmul(
                        pv[0:33, hd * 512:(hd + 1) * 512],
                        lhsT=vsb[:, m, 33 * h: 33 * h + 33],
                        rhs=pT[:, hd * 512:(hd + 1) * 512],
                        start=(m == 0),
                        stop=(m == 7),
                    )
                if m == 7:
                    pc = tmp_p.tile([128, 1024], F32R, tag="pc", name="pc")
                    nc.vector.tensor_copy(pc[0:33, :], pv[0:33, :])
                    pending_norm = (half, hA, hB, pc)
                    del pvt[pid]

            for half in range(2):
                for ip, (hA, hB) in enumerate(PAIRS_H[half]):
                    pid = half * 4 + ip
                    lag = 3 if pid == 0 else 2
                    for m in range(8):
                        st = pst.tile([128, 1024], F32, tag="ps")
                        for hd, h in ((0, hA), (1, hB)):
                            a = 32 * (h % 4)
                            hc = h // 4
                            nc.tensor.matmul(
                                st[:, hd * 512:(hd + 1) * 512],
                                lhsT=kT[a:a + 32, hc, m * 128:(m + 1) * 128],
                                rhs=qT[a:a + 32, hc, half * HALF: half * HALF + 512],
                                start=True,
                                stop=True,
                                tile_position=(a, 0),
                            )
                        # drain up to two prev-pair PVs per step (so the pv
                        # evacuation lands >= 2 steps before this pair's m==0
                        # PV reuses the psum slot), plus one own-pair PV once
                        # past the lag
                        for _ in range(2):
                            if pend and pend[0][0] != pid:
                                emit_pv(*pend.pop(0))
                        if pend and pend[0][0] == pid and len(pend) > lag:
                            emit_pv(*pend.pop(0))
                        if pending_norm is not None and m >= 2:
                            emit_norm_b(*pending_norm)
                            pending_norm = None
                        pT = ppool.tile([128, 1024], F32R, tag="pT")
                        nc.scalar.activation(pT, st, AF.Exp, bias=zerob_sb, scale=SCALE)
                        pair_extra(half, ip, m)
                        pend.append((pid, half, hA, hB, m, pT))
            # ---- tail: drain the PV backlog; the last pair's (m==7) PVs are
            # followed per-head by a short normalize chain (copy, broadcast,
            # reciprocal, multiply -- heads 0 and 4 both land on rows 0:32 so
            # there is no repositioning), then the half-1 projection finishes
            # with the two K=32 row-0 contributions per tile and stores.
            while len(pend) > 1:
                emit_pv(*pend.pop(0))
            lpid, lhalf, lhA, lhB, lm, lpT = pend.pop(0)
            pv = pvt[lpid]
            nofs = lhalf * HALF
            for hd, h in ((0, lhA), (1, lhB)):
                nc.tensor.matmul(
                    pv[0:33, hd * 512:(hd + 1) * 512],
                    lhsT=vsb[:, lm, 33 * h: 33 * h + 33],
                    rhs=lpT[:, hd * 512:(hd + 1) * 512],
                    start=False,
                    stop=True,
                )
            # evacuate the two heads' unnormalized tiles + sums, one on the
            # (now idle) ScalarE and one on DVE so they overlap; the deferred
            # m==7 partial-proj evacuation is emitted AFTER the reciprocal so
            # the normalize chain jumps ahead of it in the DVE queue.
            pc2 = tmp_p.tile([128, 1024], F32R, tag="pc", name="pc2")
            nc.scalar.copy(pc2[0:33, 0:512], pv[0:33, 0:512])
            nc.scalar.copy(pc2[0:33, 512:1024], pv[0:33, 512:1024])
            # broadcast both heads' sums, ONE reciprocal over the combined
            # rows, then normalize (both heads land on rows 0:32)
            bcp2 = pst.tile([128, 1024], F32, tag="ps", name="bcp2")
            for hd in range(2):
                nc.tensor.matmul(
                    bcp2[0:32, hd * 512:(hd + 1) * 512],
                    lhsT=onesp_sb[32:33, :],
                    rhs=pc2[32:33, hd * 512:(hd + 1) * 512],
                    start=True,
                    stop=True,
                )
            rs2t = rs_p.tile([128, 1024], F32, tag="rs", name="rs2t")
            nc.vector.reciprocal(rs2t[0:32, :], bcp2[0:32, :])
            nc.vector.tensor_copy(partial4[:, 3, :], opsp7[0][:, 0:256])
            pc2b = pc2.bitcast(F32)
            for hd, h in ((0, lhA), (1, lhB)):
                nc.vector.tensor_mul(
                    attnT[0:32, h // 4, nofs:nofs + HALF],
                    pc2b[0:32, hd * 512:(hd + 1) * 512],
                    rs2t[0:32, hd * 512:(hd + 1) * 512],
                )

            if debug_dump:
                nc.sync.dma_start(dbg["d_yT"], yT.bitcast(F32))
                nc.sync.dma_start(dbg["d_qT"], qT.bitcast(F32))
                nc.sync.dma_start(dbg["d_kT"], kT.bitcast(F32))
                nc.sync.dma_start(dbg["d_v"], vsb.bitcast(F32))
                nc.sync.dma_start(dbg["d_attnT"], attnT.bitcast(F32))

            # per psum tile (two output tiles each, separate banks): fold
            # the staged partial via an identity matmul, add the row-0
            # contributions, evacuate + store per half so the A store
            # launches while B's matmuls still run
            opsf = [pst.tile([128, 1024], F32, tag="ps", name="opsfA"),
                    paux.tile([128, 1024], F32, tag="aux", name="opsfB")]
            for half_t, q in ((0, nc.sync), (1, nc.scalar)):
                for i in (0, 1):
                    nt = 4 + half_t * 2 + i
                    sl = opsf[half_t][:, i * 512:i * 512 + 256]
                    nc.tensor.matmul(
                        sl,
                        lhsT=id_sb,
                        rhs=partial4[:, half_t * 2 + i, :],
                        start=True,
                        stop=False,
                    )
                    for chunk in range(2):
                        nc.tensor.matmul(
                            sl,
                            lhsT=attnT[0:32, chunk, nt * 128:(nt + 1) * 128],
                            rhs=outwT_sb[0:32, chunk, :],
                            start=False,
                            stop=(chunk == 1),
                            tile_position=(0, 0),
                        )
                osb2 = outs_p.tile([128, 2, C], F32, tag="o", name="osb2")
                osrc = opsf[half_t].rearrange("p (t c) -> p t c", t=2)[:, :, 0:256]
                if half_t == 0:
                    nc.vector.tensor_copy(osb2, osrc)
                else:
                    nc.scalar.copy(osb2, osrc)
                r0 = 512 + half_t * 256
                q.dma_start(
                    out_d[r0:r0 + 256, :].rearrange("(t p) c -> p t c", p=128),
                    osb2,
                )

    nc.compile()
    return nc


_NC = None
LAST_RESULTS = None


def _host_prep(conv_w, conv_b, qkv_w, out_w, out_b):
    conv_w = np.asarray(conv_w, np.float32).reshape(C, 3, 3)
    taps = np.zeros((128, 18), np.float32)
    for ct in range(2):
        for t, (ky, kx) in enumerate(TAPS):
            d = conv_w[128 * ct: 128 * (ct + 1), ky, kx].copy()
            if (ky, kx) == (1, 1):
                d += 1.0  # residual connection folded into the center tap
            taps[:, ct * 9 + t] = d
    return {
        "qkv_wT": np.ascontiguousarray(np.asarray(qkv_w, np.float32).T),
        "out_wT": np.ascontiguousarray(np.asarray(out_w, np.float32).T),
        "conv_taps": taps,
        "conv_b_r": np.asarray(conv_b, np.float32).reshape(1, C),
        "out_b_r": np.asarray(out_b, np.float32).reshape(1, C),
        "id128": np.eye(128, dtype=np.float32),
    }


def kernel(x, conv_w, conv_b, qkv_w, out_w, out_b):
    global _NC, LAST_RESULTS
    if _NC is None:
        _NC = build_nc()
    x = np.asarray(x, np.float32)
    shared = _host_prep(conv_w, conv_b, qkv_w, out_w, out_b)
    in_maps = [{**shared, "x": np.ascontiguousarray(x[b])} for b in range(B)]
    trace = bool(int(os.environ.get("KERNEL_TRACE", "0")))
    try:
        res = run_bass_kernel_spmd(_NC, in_maps, core_ids=list(range(B)), trace=trace)
    except Exception:
        if not trace:
            raise
        res = run_bass_kernel_spmd(_NC, in_maps, core_ids=list(range(B)), trace=False)
    LAST_RESULTS = res
    return np.stack([res.results[b]["out"] for b in range(B)], axis=0)
